# revision 12
# baseline (speedup 1.0000x reference)
"""FAGCN layer on 8 Trainium2 NeuronCores (Bass/Tile).

Strategy (1D graph partition, dst-sharded):
  - Host: relabel nodes into 1568 degree-balanced windows of 64 slots
    (196 windows per core); bucket edges by dst window; split each
    window's edges by src slab (2 slabs reachable via int16 gather
    indices against base-biased table views).
  - Launch 1 (dense, node-sharded): h = relu(x@w1T+b1), gate scalars
    a_dst/a_src = h@gwT, norm = clip(deg,1)^-1/2. All-f16 matmuls on
    host-pretransposed xT (hid-major PSUM, no input transposes), deg
    preloaded transposed, outputs staged in SBUF and written in 7-block
    groups. Emits a gather table (f16: hn=norm*h, a_src hi/lo) and a
    per-core own-shard table (f32: 0.3*h, a_dst, norm).
  - Host: all-gather the f16 table (pure concatenation).
  - Launch 2 (edge phase, dst-sharded): per 128-edge chunk, dma_gather
    hn rows by src; per-edge gate g=tanh(a_dst[dst]+a_src[src]+gb) with
    a_dst broadcast via host-built one-hot matvec on the PE; scatter-add
    via one-hot matmul into PSUM per 64-dst window; drain applies
    norm[dst] and the eps*h residual.
"""
import sys

if "/opt/trn_rl_repo" not in sys.path:
    sys.path.insert(0, "/opt/trn_rl_repo")

import numpy as np

from concourse import bacc, bass, mybir, tile
from concourse.bass_utils import run_bass_kernel_spmd
from concourse.masks import make_identity

f32 = mybir.dt.float32
f16 = mybir.dt.float16
i16 = mybir.dt.int16
i32 = mybir.dt.int32
AF = mybir.ActivationFunctionType

N = 100000
E = 1600000
IN_DIM = 256
HID = 128
EPS = 0.3

NCORES = 8
WIN = 64
NWT = 1568            # total windows
NWC = NWT // NCORES   # 196 windows per core
NPC = NWC * WIN       # 12544 slots per core
NSLOT = NWT * WIN     # 100352 total slots
SB = 4                # windows per superblock
NSB = NWC // SB       # 49 superblocks per core
SPLIT = 56448         # slab A = slots [0, SPLIT); must be mult of 64
BA = SPLIT - 32768    # base row of slab-A view
BB = SPLIT + 32768    # base row of slab-B view
EW = 256              # f16 elements per gather row (512B)

_prog_cache = {}
LAST_EXEC_NS = None  # [phase1_ns, phase2_ns] when KERNEL_TRACE=1


def _build_phase1():
    GB = 7  # blocks per write group (98 = 14*7)
    nc = bacc.Bacc(None)
    xT = nc.dram_tensor("xT", [2, 128, NPC], f16, kind="ExternalInput")
    deg = nc.dram_tensor("deg", [NPC, 1], f32, kind="ExternalInput")
    w1T = nc.dram_tensor("w1T", [2, 128, HID], f16, kind="ExternalInput")
    b1 = nc.dram_tensor("b1", [HID, 1], f32, kind="ExternalInput")
    gwT = nc.dram_tensor("gwT", [HID, 2], f16, kind="ExternalInput")
    aug = nc.dram_tensor("aug", [NPC, EW], f16, kind="ExternalOutput")
    own = nc.dram_tensor("own", [NPC, 132], f32, kind="ExternalOutput")

    with tile.TileContext(nc) as tc:
        with (
            tc.tile_pool(name="const", bufs=1) as cp,
            tc.tile_pool(name="sb", bufs=3) as sb,
            tc.tile_pool(name="wr", bufs=2) as wr,
            tc.tile_pool(name="ps", bufs=2, space="PSUM") as ps,
        ):
            identf = cp.tile([128, 128], f32)
            make_identity(nc, identf[:])
            ident = cp.tile([128, 128], f16)
            nc.vector.tensor_copy(ident[:], identf[:])
            xT_t = [cp.tile([128, NPC], f16, tag=f"xT{k}", name=f"xT{k}") for k in range(2)]
            for k in range(2):
                nc.sync.dma_start(out=xT_t[k][:], in_=xT[k])
            w1T_t = [cp.tile([128, HID], f16, tag=f"w1T{k}", name=f"w1T{k}") for k in range(2)]
            for k in range(2):
                nc.sync.dma_start(out=w1T_t[k][:], in_=w1T[k])
            b1_t = cp.tile([HID, 1], f32)
            nc.sync.dma_start(out=b1_t[:], in_=b1[:, :])
            gw_t = cp.tile([HID, 2], f16)
            nc.sync.dma_start(out=gw_t[:], in_=gwT[:, :])
            degT = cp.tile([128, NPC // 128], f32)
            nc.sync.dma_start(
                out=degT[:], in_=deg.rearrange("(b p) o -> p (b o)", p=128)
            )

            for g0 in range(0, NPC // 128, GB):
                augb = wr.tile([128, GB, EW], f16, tag="augb")
                ownb = wr.tile([128, GB, 132], f32, tag="ownb")
                for j in range(GB):
                    blk = g0 + j
                    r0 = blk * 128
                    hT_ps = ps.tile([128, 128], f32, tag="hT_ps")
                    for k in range(2):
                        nc.tensor.matmul(
                            out=hT_ps[:], lhsT=w1T_t[k][:],
                            rhs=xT_t[k][:, r0 : r0 + 128],
                            start=(k == 0), stop=(k == 1),
                        )
                    hT = sb.tile([128, 128], f16, tag="hT")
                    nc.scalar.activation(
                        out=hT[:], in_=hT_ps[:], func=AF.Relu, bias=b1_t[:]
                    )
                    ga_ps = ps.tile([128, 2], f32, tag="ga_ps")
                    nc.tensor.matmul(
                        out=ga_ps[:], lhsT=hT[:], rhs=gw_t[:], start=True, stop=True
                    )
                    hn_ps = ps.tile([128, 128], f16, tag="hn_ps")
                    nc.tensor.transpose(hn_ps[:], hT[:], ident[:])

                    dc = sb.tile([128, 1], f32, tag="dc")
                    nc.vector.tensor_scalar_max(dc[:], degT[:, blk : blk + 1], 1.0)
                    sq = sb.tile([128, 1], f32, tag="sq")
                    nc.scalar.sqrt(sq[:], dc[:])
                    nrm = sb.tile([128, 1], f32, tag="nrm")
                    nc.vector.reciprocal(nrm[:], sq[:])

                    # own: 0.3*h (0:128), a_dst (128), norm (129)
                    nc.vector.tensor_scalar_mul(ownb[:, j, 0:128], hn_ps[:], EPS)
                    nc.vector.tensor_copy(ownb[:, j, 128:129], ga_ps[:, 0:1])
                    nc.vector.tensor_copy(ownb[:, j, 129:130], nrm[:])
                    nc.vector.memset(ownb[:, j, 130:132], 0.0)

                    # aug: hn=norm*h (0:128), a_src hi (128), lo (129)
                    nc.scalar.activation(
                        out=augb[:, j, 0:128], in_=hn_ps[:], func=AF.Copy, scale=nrm[:]
                    )
                    hi16 = sb.tile([128, 1], f16, tag="hi16")
                    nc.vector.tensor_copy(hi16[:], ga_ps[:, 1:2])
                    hi32 = sb.tile([128, 1], f32, tag="hi32")
                    nc.vector.tensor_copy(hi32[:], hi16[:])
                    lo32 = sb.tile([128, 1], f32, tag="lo32")
                    nc.vector.tensor_sub(lo32[:], ga_ps[:, 1:2], hi32[:])
                    nc.vector.tensor_copy(augb[:, j, 128:129], hi16[:])
                    nc.vector.tensor_copy(augb[:, j, 129:130], lo32[:])
                    nc.vector.memset(augb[:, j, 130:EW], 0.0)
                nc.sync.dma_start(
                    out=aug[g0 * 128 : (g0 + GB) * 128, :].rearrange(
                        "(g p) e -> p g e", p=128
                    ),
                    in_=augb[:],
                )
                nc.sync.dma_start(
                    out=own[g0 * 128 : (g0 + GB) * 128, :].rearrange(
                        "(g p) e -> p g e", p=128
                    ),
                    in_=ownb[:],
                )
    nc.finalize()
    return nc


def _build_phase2(CA, CB):
    import os
    NO_GATHER = os.environ.get("P2_NO_GATHER", "0") == "1"
    NO_MATVEC = os.environ.get("P2_NO_MATVEC", "0") == "1"
    NO_SW = os.environ.get("P2_NO_SW", "0") == "1"
    NO_SCATTER = os.environ.get("P2_NO_SCATTER", "0") == "1"
    K = CA + CB
    NIA = 128 * SB * CA
    NIB = 128 * SB * CB
    nc = bacc.Bacc(None, dynamic_dma_scratch_size=65536)
    aug = nc.dram_tensor("aug", [NSLOT, EW], f16, kind="ExternalInput")
    own = nc.dram_tensor("own", [NPC, 132], f32, kind="ExternalInput")
    ia = nc.dram_tensor("ia", [NSB, 128, NIA // 16], i16, kind="ExternalInput")
    ib = nc.dram_tensor("ib", [NSB, 128, NIB // 16], i16, kind="ExternalInput")
    dl = nc.dram_tensor("dl", [NSB, 128, SB * K], f32, kind="ExternalInput")
    s2 = nc.dram_tensor("s2", [NSB, 64, SB * K * 128], f16, kind="ExternalInput")
    gbc = nc.dram_tensor("gbc", [128, 1], f32, kind="ExternalInput")
    out = nc.dram_tensor("out", [NPC, HID], f32, kind="ExternalOutput")

    with tile.TileContext(nc) as tc:
        with (
            tc.tile_pool(name="const", bufs=1) as cp,
            tc.tile_pool(name="gpool", bufs=4) as gp,
            tc.tile_pool(name="sbp", bufs=3) as sbp,
            tc.tile_pool(name="swp", bufs=4) as swp,
            tc.tile_pool(name="psz", bufs=2, space="PSUM") as psz,
            tc.tile_pool(name="psa", bufs=2, space="PSUM") as psa,
        ):
            iota_i = cp.tile([128, WIN], i32)
            nc.gpsimd.iota(iota_i[:], pattern=[[1, WIN]], base=0, channel_multiplier=0)
            iota_f = cp.tile([128, WIN], f16)
            nc.vector.tensor_copy(iota_f[:], iota_i[:])
            gb_t = cp.tile([128, 1], f32)
            nc.sync.dma_start(out=gb_t[:], in_=gbc[:, :])

            for sbi in range(NSB):
                ita = sbp.tile([128, NIA // 16], i16, tag="ita")
                nc.sync.dma_start(out=ita[:], in_=ia[sbi])
                itb = sbp.tile([128, NIB // 16], i16, tag="itb")
                nc.sync.dma_start(out=itb[:], in_=ib[sbi])
                dlt = sbp.tile([128, SB * K], f32, tag="dlt")
                nc.sync.dma_start(out=dlt[:], in_=dl[sbi])
                dls = sbp.tile([128, SB * K], f32, tag="dls")
                nc.vector.tensor_copy(dls[:], dlt[:])
                s2t = sbp.tile([64, SB * K * 128], f16, tag="s2t")
                nc.sync.dma_start(out=s2t[:], in_=s2[sbi])

                GA = gp.tile([128, SB * CA, EW], f16, tag="GA")
                GB = gp.tile([128, SB * CB, EW], f16, tag="GB")
                if NO_GATHER:
                    nc.vector.memset(GA[:], 0.0)
                    nc.vector.memset(GB[:], 0.0)
                else:
                    # per-call cap: 1024 idxs (64 descriptors per SWDGE stripe)
                    for Gt, view_base, it, L in (
                        (GA, BA, ita, NIA),
                        (GB, BB, itb, NIB),
                    ):
                        o = 0
                        while o < L:
                            n = min(1024, L - o)
                            nc.gpsimd.dma_gather(
                                Gt[:, o // 128 : (o + n) // 128, :],
                                aug[view_base:, :],
                                it[:, o // 16 : (o + n) // 16],
                                n,
                                n,
                                EW,
                            )
                            o += n

                for wl in range(SB):
                    w = sbi * SB + wl
                    ow = sbp.tile([WIN, 132], f32, tag="ow")
                    nc.sync.dma_start(out=ow[:], in_=own[w * WIN : (w + 1) * WIN, :])
                    adw = sbp.tile([WIN, 1], f16, tag="adw")
                    nc.scalar.activation(out=adw[:], in_=ow[:, 128:129], func=AF.Copy)

                    adp = psa.tile([128, K], f32, tag="adp")
                    if NO_MATVEC:
                        nc.vector.memset(adp[:], 0.0)
                    else:
                        for c in range(K):
                            j = wl * K + c
                            nc.tensor.matmul(
                                out=adp[:, c : c + 1],
                                lhsT=s2t[:, j * 128 : (j + 1) * 128],
                                rhs=adw[:],
                                start=True,
                                stop=True,
                            )
                    tt = sbp.tile([128, K], f32, tag="tt")
                    nc.vector.tensor_tensor(
                        out=tt[:, 0:CA],
                        in0=GA[:, wl * CA : (wl + 1) * CA, 128],
                        in1=GA[:, wl * CA : (wl + 1) * CA, 129],
                        op=mybir.AluOpType.add,
                    )
                    nc.vector.tensor_tensor(
                        out=tt[:, CA:K],
                        in0=GB[:, wl * CB : (wl + 1) * CB, 128],
                        in1=GB[:, wl * CB : (wl + 1) * CB, 129],
                        op=mybir.AluOpType.add,
                    )
                    t1 = sbp.tile([128, K], f32, tag="t1")
                    nc.vector.tensor_tensor(
                        out=t1[:], in0=tt[:], in1=adp[:], op=mybir.AluOpType.add
                    )
                    g = sbp.tile([128, K], f32, tag="g")
                    nc.scalar.activation(out=g[:], in_=t1[:], func=AF.Tanh, bias=gb_t[:])
                    gs = sbp.tile([128, K], f32, tag="gs")
                    nc.vector.tensor_copy(gs[:], g[:])

                    zp = psz.tile([WIN, 128], f32, tag="zp")
                    for c in range(K):
                        j = wl * K + c
                        if c < CA:
                            Gx, col = GA, wl * CA + c
                        else:
                            Gx, col = GB, wl * CB + (c - CA)
                        sw = iota_f if NO_SW else swp.tile([128, WIN], f16, tag="sw")
                        if not NO_SW:
                            nc.vector.tensor_scalar(
                                out=sw[:],
                                in0=iota_f[:],
                                scalar1=dls[:, j : j + 1],
                                scalar2=gs[:, c : c + 1],
                                op0=mybir.AluOpType.is_equal,
                                op1=mybir.AluOpType.mult,
                            )
                        if not NO_SCATTER:
                            nc.tensor.matmul(
                                out=zp[:],
                                lhsT=sw[:],
                                rhs=Gx[:, col, 0:128],
                                start=(c == 0),
                                stop=(c == K - 1),
                            )
                        elif c == 0:
                            nc.vector.memset(zp[:], 0.0)
                    d1 = sbp.tile([WIN, 128], f32, tag="d1")
                    nc.scalar.activation(
                        out=d1[:], in_=zp[:], func=AF.Copy, scale=ow[:, 129:130]
                    )
                    ot = sbp.tile([WIN, 128], f32, tag="ot")
                    nc.vector.tensor_add(ot[:], d1[:], ow[:, 0:128])
                    nc.sync.dma_start(out=out[w * WIN : (w + 1) * WIN, :], in_=ot[:])
    nc.finalize()
    return nc


def _pack_idx16(flat_rel, n):
    """flat order i -> [i%16, i//16]; replicated 8x across 128 partitions."""
    t = np.zeros((16, n // 16), np.int16)
    t[np.arange(n) % 16, np.arange(n) // 16] = flat_rel
    return np.tile(t, (8, 1))


def _host_prep(edge_index):
    src = edge_index[0].astype(np.int64)
    dst = edge_index[1].astype(np.int64)
    deg = np.bincount(dst, minlength=N).astype(np.int64)

    # degree-balanced window assignment (iterative LPT)
    order = np.argsort(-deg, kind="stable")
    win_of_node = np.empty(N, np.int64)
    pos_of_node = np.empty(N, np.int64)
    load = np.zeros(NWT, np.int64)
    cnt = np.zeros(NWT, np.int64)
    for r0 in range(0, N, NWT):
        grp = order[r0 : r0 + NWT]
        bins = np.argsort(load, kind="stable")[: len(grp)]
        win_of_node[grp] = bins
        pos_of_node[grp] = cnt[bins]
        cnt[bins] += 1
        load[bins] += deg[grp]

    slot_of_node = win_of_node * WIN + pos_of_node
    orig_of_slot = np.full(NSLOT, -1, np.int64)
    orig_of_slot[slot_of_node] = np.arange(N)

    # per-edge attributes
    ew = win_of_node[dst]                # window
    es = slot_of_node[src]               # src slot
    eslab = (es >= SPLIT).astype(np.int64)
    edl = pos_of_node[dst]               # dst position in window

    key = ew * 2 + eslab
    sidx = np.argsort(key, kind="stable")
    key_s = key[sidx]
    counts = np.bincount(key_s, minlength=NWT * 2)
    gstart = np.zeros(NWT * 2, np.int64)
    gstart[1:] = np.cumsum(counts)[:-1]
    qpos = np.arange(E) - gstart[key_s]  # position within (window, slab) group

    cntsA = counts[0::2]
    cntsB = counts[1::2]
    CA = int(np.ceil(cntsA.max() / 128))
    CB = int(np.ceil(cntsB.max() / 128))
    K = CA + CB

    es_s = es[sidx]
    edl_s = edl[sidx]
    ew_s = ew[sidx]
    eslab_s = eslab[sidx]

    core_s = ew_s // NWC
    wloc = ew_s % NWC
    sb_s = wloc // SB
    wl_s = wloc % SB
    p_s = qpos % 128
    c_loc = qpos // 128
    cchunk = np.where(eslab_s == 0, c_loc, CA + c_loc)
    j_s = wl_s * K + cchunk
    relidx = np.where(eslab_s == 0, es_s - BA, es_s - BB)

    NIA = 128 * SB * CA
    NIB = 128 * SB * CB
    per_core = []
    for c in range(NCORES):
        m = core_s == c
        sbv, wlv, pv, clv, jv = sb_s[m], wl_s[m], p_s[m], c_loc[m], j_s[m]
        slabv, relv, dlv, qv = eslab_s[m], relidx[m], edl_s[m], qpos[m]

        dl_arr = np.full((NSB, 128, SB * K), -1.0, np.float32)
        dl_arr[sbv, pv, jv] = dlv
        s2_arr = np.zeros((NSB, 64, SB * K * 128), np.float16)
        s2_arr[sbv, dlv, jv * 128 + pv] = 1.0

        fa = np.zeros((NSB, NIA), np.int64)   # rel idx 0 = valid pad row
        fb = np.zeros((NSB, NIB), np.int64)
        mA = slabv == 0
        fa[sbv[mA], wlv[mA] * CA * 128 + qv[mA]] = relv[mA]
        mB = ~mA
        fb[sbv[mB], wlv[mB] * CB * 128 + qv[mB]] = relv[mB]

        # The Q7 gather drops a call's trailing run of negative indices, so
        # every call's final slot must hold a non-negative index. Swap a
        # non-negative slot from the same (window, slab) group into each
        # static call-tail position.
        def _fix_tails(f, C, joff):
            L = f.shape[1]
            tails = [min(o + 1024, L) - 1 for o in range(0, L, 1024)]
            span = 128 * C
            for s in range(NSB):
                for t in tails:
                    if f[s, t] >= 0:
                        continue
                    wg = t // span
                    g0 = wg * span
                    seg = f[s, g0 : g0 + span]
                    cand = np.nonzero(seg >= 0)[0]
                    assert cand.size, "all-negative gather group"
                    u = g0 + int(cand[-1])
                    p_t, c_t = t % 128, (t // 128) % C
                    p_u, c_u = u % 128, (u // 128) % C
                    j_t = wg * K + joff + c_t
                    j_u = wg * K + joff + c_u
                    f[s, t], f[s, u] = f[s, u], f[s, t]
                    tmp = dl_arr[s, p_t, j_t]
                    dl_arr[s, p_t, j_t] = dl_arr[s, p_u, j_u]
                    dl_arr[s, p_u, j_u] = tmp
                    ct, cu = j_t * 128 + p_t, j_u * 128 + p_u
                    tcol = s2_arr[s, :, ct].copy()
                    s2_arr[s, :, ct] = s2_arr[s, :, cu]
                    s2_arr[s, :, cu] = tcol

        _fix_tails(fa, CA, 0)
        _fix_tails(fb, CB, CA)
        ia_arr = np.stack([_pack_idx16(fa[s], NIA) for s in range(NSB)])
        ib_arr = np.stack([_pack_idx16(fb[s], NIB) for s in range(NSB)])
        per_core.append(dict(ia=ia_arr, ib=ib_arr, dl=dl_arr, s2=s2_arr))

    return dict(
        deg=deg, orig_of_slot=orig_of_slot, slot_of_node=slot_of_node,
        CA=CA, CB=CB, per_core=per_core,
    )


def kernel(x, edge_index, w1, b1, gate_w, gate_b):
    x = np.asarray(x, np.float32)
    edge_index = np.asarray(edge_index)
    w1 = np.asarray(w1, np.float32)
    b1 = np.asarray(b1, np.float32)
    gate_w = np.asarray(gate_w, np.float32)
    gate_b = np.asarray(gate_b, np.float32)

    prep = _host_prep(edge_index)
    CA, CB = prep["CA"], prep["CB"]
    orig_of_slot = prep["orig_of_slot"]
    deg = prep["deg"]

    # per-slot x / deg (zeros for empty slots)
    x_slots = np.zeros((NSLOT, IN_DIM), np.float32)
    deg_slots = np.zeros((NSLOT, 1), np.float32)
    valid = orig_of_slot >= 0
    x_slots[valid] = x[orig_of_slot[valid]]
    deg_slots[valid, 0] = deg[orig_of_slot[valid]]

    if "p1" not in _prog_cache:
        _prog_cache["p1"] = _build_phase1()
    nc1 = _prog_cache["p1"]

    w1T = np.ascontiguousarray(w1.T.reshape(2, 128, HID).astype(np.float16))
    b1c = np.ascontiguousarray(b1[:, None])
    gwT = np.ascontiguousarray(gate_w.reshape(2, HID).T.astype(np.float16))
    in_maps1 = [
        dict(
            xT=np.ascontiguousarray(
                x_slots[c * NPC : (c + 1) * NPC].T.reshape(2, 128, NPC)
            ).astype(np.float16),
            deg=deg_slots[c * NPC : (c + 1) * NPC],
            w1T=w1T, b1=b1c, gwT=gwT,
        )
        for c in range(NCORES)
    ]
    import os

    do_trace = os.environ.get("KERNEL_TRACE", "0") == "1"
    global LAST_EXEC_NS
    LAST_EXEC_NS = [None, None]
    br1 = run_bass_kernel_spmd(nc1, in_maps1, list(range(NCORES)), trace=do_trace)
    r1 = br1.results
    LAST_EXEC_NS[0] = br1.exec_time_ns

    aug_full = np.concatenate([r1[c]["aug"] for c in range(NCORES)], axis=0)

    key2 = ("p2", CA, CB)
    if key2 not in _prog_cache:
        _prog_cache[key2] = _build_phase2(CA, CB)
    nc2 = _prog_cache[key2]

    gbc = np.full((128, 1), float(gate_b[0]), np.float32)
    in_maps2 = [
        dict(
            aug=aug_full,
            own=r1[c]["own"],
            ia=prep["per_core"][c]["ia"],
            ib=prep["per_core"][c]["ib"],
            dl=prep["per_core"][c]["dl"],
            s2=prep["per_core"][c]["s2"],
            gbc=gbc,
        )
        for c in range(NCORES)
    ]
    br2 = run_bass_kernel_spmd(nc2, in_maps2, list(range(NCORES)), trace=do_trace)
    r2 = br2.results
    LAST_EXEC_NS[1] = br2.exec_time_ns

    out_slots = np.concatenate([r2[c]["out"] for c in range(NCORES)], axis=0)
    result = np.empty((N, HID), np.float32)
    result[orig_of_slot[valid]] = out_slots[valid]
    return result



# revision 13
# speedup vs baseline: 1.1405x; 1.1405x over previous
"""FAGCN layer on 8 Trainium2 NeuronCores (Bass/Tile).

Strategy (1D graph partition, dst-sharded):
  - Host: relabel nodes into 1568 degree-balanced windows of 64 slots
    (196 windows per core); bucket edges by dst window; split each
    window's edges by src slab (2 slabs reachable via int16 gather
    indices against base-biased table views).
  - Launch 1 (dense, node-sharded): h = relu(x@w1T+b1), gate scalars
    a_dst/a_src = h@gwT, norm = clip(deg,1)^-1/2. All-f16 matmuls on
    host-pretransposed xT (hid-major PSUM, no input transposes), deg
    preloaded transposed, outputs staged in SBUF and written in 7-block
    groups. Emits a gather table (f16: hn=norm*h, a_src hi/lo) and a
    per-core own-shard table (f32: 0.3*h, a_dst, norm).
  - Host: all-gather the f16 table (pure concatenation).
  - Launch 2 (edge phase, dst-sharded): per 128-edge chunk, dma_gather
    hn rows by src; per-edge gate g=tanh(a_dst[dst]+a_src[src]+gb) with
    a_dst broadcast via host-built one-hot matvec on the PE; scatter-add
    via one-hot matmul into PSUM per 64-dst window; drain applies
    norm[dst] and the eps*h residual.
"""
import sys

if "/opt/trn_rl_repo" not in sys.path:
    sys.path.insert(0, "/opt/trn_rl_repo")

import numpy as np

from concourse import bacc, bass, mybir, tile
from concourse.bass_utils import run_bass_kernel_spmd
from concourse.masks import make_identity

f32 = mybir.dt.float32
f16 = mybir.dt.float16
i16 = mybir.dt.int16
i32 = mybir.dt.int32
AF = mybir.ActivationFunctionType

N = 100000
E = 1600000
IN_DIM = 256
HID = 128
EPS = 0.3

NCORES = 8
WIN = 64
NWT = 1568            # total windows
NWC = NWT // NCORES   # 196 windows per core
NPC = NWC * WIN       # 12544 slots per core
NSLOT = NWT * WIN     # 100352 total slots
SB = 4                # windows per superblock
NSB = NWC // SB       # 49 superblocks per core
SPLIT = 56448         # slab A = slots [0, SPLIT); must be mult of 64
BA = SPLIT - 32768    # base row of slab-A view
BB = SPLIT + 32768    # base row of slab-B view
EW = 256              # f16 elements per gather row (512B)

_prog_cache = {}
LAST_EXEC_NS = None  # [phase1_ns, phase2_ns] when KERNEL_TRACE=1


def _build_phase1():
    GB = 7  # blocks per write group (98 = 14*7)
    nc = bacc.Bacc(None)
    xT = nc.dram_tensor("xT", [2, 128, NPC], f16, kind="ExternalInput")
    deg = nc.dram_tensor("deg", [NPC, 1], f32, kind="ExternalInput")
    w1T = nc.dram_tensor("w1T", [2, 128, HID], f16, kind="ExternalInput")
    b1 = nc.dram_tensor("b1", [HID, 1], f32, kind="ExternalInput")
    gwT = nc.dram_tensor("gwT", [HID, 2], f16, kind="ExternalInput")
    aug = nc.dram_tensor("aug", [NPC, EW], f16, kind="ExternalOutput")
    own = nc.dram_tensor("own", [NPC, 132], f32, kind="ExternalOutput")

    with tile.TileContext(nc) as tc:
        with (
            tc.tile_pool(name="const", bufs=1) as cp,
            tc.tile_pool(name="sb", bufs=3) as sb,
            tc.tile_pool(name="wr", bufs=2) as wr,
            tc.tile_pool(name="ps", bufs=2, space="PSUM") as ps,
        ):
            identf = cp.tile([128, 128], f32)
            make_identity(nc, identf[:])
            ident = cp.tile([128, 128], f16)
            nc.vector.tensor_copy(ident[:], identf[:])
            xT_t = [cp.tile([128, NPC], f16, tag=f"xT{k}", name=f"xT{k}") for k in range(2)]
            for k in range(2):
                nc.sync.dma_start(out=xT_t[k][:], in_=xT[k])
            w1T_t = [cp.tile([128, HID], f16, tag=f"w1T{k}", name=f"w1T{k}") for k in range(2)]
            for k in range(2):
                nc.sync.dma_start(out=w1T_t[k][:], in_=w1T[k])
            b1_t = cp.tile([HID, 1], f32)
            nc.sync.dma_start(out=b1_t[:], in_=b1[:, :])
            gw_t = cp.tile([HID, 2], f16)
            nc.sync.dma_start(out=gw_t[:], in_=gwT[:, :])
            degT = cp.tile([128, NPC // 128], f32)
            nc.sync.dma_start(
                out=degT[:], in_=deg.rearrange("(b p) o -> p (b o)", p=128)
            )

            for g0 in range(0, NPC // 128, GB):
                augb = wr.tile([128, GB, EW], f16, tag="augb")
                ownb = wr.tile([128, GB, 132], f32, tag="ownb")
                for j in range(GB):
                    blk = g0 + j
                    r0 = blk * 128
                    hT_ps = ps.tile([128, 128], f32, tag="hT_ps")
                    for k in range(2):
                        nc.tensor.matmul(
                            out=hT_ps[:], lhsT=w1T_t[k][:],
                            rhs=xT_t[k][:, r0 : r0 + 128],
                            start=(k == 0), stop=(k == 1),
                        )
                    hT = sb.tile([128, 128], f16, tag="hT")
                    nc.scalar.activation(
                        out=hT[:], in_=hT_ps[:], func=AF.Relu, bias=b1_t[:]
                    )
                    ga_ps = ps.tile([128, 2], f32, tag="ga_ps")
                    nc.tensor.matmul(
                        out=ga_ps[:], lhsT=hT[:], rhs=gw_t[:], start=True, stop=True
                    )
                    hn_ps = ps.tile([128, 128], f16, tag="hn_ps")
                    nc.tensor.transpose(hn_ps[:], hT[:], ident[:])

                    dc = sb.tile([128, 1], f32, tag="dc")
                    nc.vector.tensor_scalar_max(dc[:], degT[:, blk : blk + 1], 1.0)
                    sq = sb.tile([128, 1], f32, tag="sq")
                    nc.scalar.sqrt(sq[:], dc[:])
                    nrm = sb.tile([128, 1], f32, tag="nrm")
                    nc.vector.reciprocal(nrm[:], sq[:])

                    # own: 0.3*h (0:128), a_dst (128), norm (129)
                    nc.vector.tensor_scalar_mul(ownb[:, j, 0:128], hn_ps[:], EPS)
                    nc.vector.tensor_copy(ownb[:, j, 128:129], ga_ps[:, 0:1])
                    nc.vector.tensor_copy(ownb[:, j, 129:130], nrm[:])
                    nc.vector.memset(ownb[:, j, 130:132], 0.0)

                    # aug: hn=norm*h (0:128), a_src hi (128), lo (129)
                    nc.scalar.activation(
                        out=augb[:, j, 0:128], in_=hn_ps[:], func=AF.Copy, scale=nrm[:]
                    )
                    hi16 = sb.tile([128, 1], f16, tag="hi16")
                    nc.vector.tensor_copy(hi16[:], ga_ps[:, 1:2])
                    hi32 = sb.tile([128, 1], f32, tag="hi32")
                    nc.vector.tensor_copy(hi32[:], hi16[:])
                    lo32 = sb.tile([128, 1], f32, tag="lo32")
                    nc.vector.tensor_sub(lo32[:], ga_ps[:, 1:2], hi32[:])
                    nc.vector.tensor_copy(augb[:, j, 128:129], hi16[:])
                    nc.vector.tensor_copy(augb[:, j, 129:130], lo32[:])
                    nc.vector.memset(augb[:, j, 130:EW], 0.0)
                nc.sync.dma_start(
                    out=aug[g0 * 128 : (g0 + GB) * 128, :].rearrange(
                        "(g p) e -> p g e", p=128
                    ),
                    in_=augb[:],
                )
                nc.sync.dma_start(
                    out=own[g0 * 128 : (g0 + GB) * 128, :].rearrange(
                        "(g p) e -> p g e", p=128
                    ),
                    in_=ownb[:],
                )
    nc.finalize()
    return nc


def _build_phase2(CA, CB):
    import os
    NO_GATHER = os.environ.get("P2_NO_GATHER", "0") == "1"
    NO_MATVEC = os.environ.get("P2_NO_MATVEC", "0") == "1"
    NO_SW = os.environ.get("P2_NO_SW", "0") == "1"
    NO_SCATTER = os.environ.get("P2_NO_SCATTER", "0") == "1"
    K = CA + CB
    NIA = 128 * SB * CA
    NIB = 128 * SB * CB
    nc = bacc.Bacc(None, dynamic_dma_scratch_size=65536)
    aug = nc.dram_tensor("aug", [NSLOT, EW], f16, kind="ExternalInput")
    own = nc.dram_tensor("own", [NPC, 132], f32, kind="ExternalInput")
    ia = nc.dram_tensor("ia", [NSB, 128, NIA // 16], i16, kind="ExternalInput")
    ib = nc.dram_tensor("ib", [NSB, 128, NIB // 16], i16, kind="ExternalInput")
    dl = nc.dram_tensor("dl", [NSB, 128, SB * K], f32, kind="ExternalInput")
    s2 = nc.dram_tensor("s2", [NSB, 64, SB * K * 128], f16, kind="ExternalInput")
    gbc = nc.dram_tensor("gbc", [128, 1], f32, kind="ExternalInput")
    out = nc.dram_tensor("out", [NPC, HID], f32, kind="ExternalOutput")

    with tile.TileContext(nc) as tc:
        with (
            tc.tile_pool(name="const", bufs=1) as cp,
            tc.tile_pool(name="gpool", bufs=3) as gp,
            tc.tile_pool(name="sbp", bufs=3) as sbp,
            tc.tile_pool(name="swp", bufs=4) as swp,
            tc.tile_pool(name="psz", bufs=2, space="PSUM") as psz,
            tc.tile_pool(name="psa", bufs=2, space="PSUM") as psa,
        ):
            iota_i = cp.tile([128, WIN], i32)
            nc.gpsimd.iota(iota_i[:], pattern=[[1, WIN]], base=0, channel_multiplier=0)
            iota_f = cp.tile([128, WIN], f16)
            nc.vector.tensor_copy(iota_f[:], iota_i[:])
            gb_t = cp.tile([128, 1], f32)
            nc.sync.dma_start(out=gb_t[:], in_=gbc[:, :])

            for sbi in range(NSB):
                ita = sbp.tile([128, NIA // 16], i16, tag="ita")
                nc.sync.dma_start(out=ita[:], in_=ia[sbi])
                itb = sbp.tile([128, NIB // 16], i16, tag="itb")
                nc.sync.dma_start(out=itb[:], in_=ib[sbi])
                dlt = sbp.tile([128, SB * K], f32, tag="dlt")
                nc.sync.dma_start(out=dlt[:], in_=dl[sbi])
                dls = sbp.tile([128, SB * K], f32, tag="dls")
                nc.vector.tensor_copy(dls[:], dlt[:])
                s2t = sbp.tile([64, SB * K * 128], f16, tag="s2t")
                nc.sync.dma_start(out=s2t[:], in_=s2[sbi])

                GA = gp.tile([128, SB * CA, EW], f16, tag="GA")
                GB = gp.tile([128, SB * CB, EW], f16, tag="GB")
                if NO_GATHER:
                    nc.vector.memset(GA[:], 0.0)
                    nc.vector.memset(GB[:], 0.0)
                else:
                    # per-call cap: 1024 idxs (64 descriptors per SWDGE stripe)
                    for Gt, view_base, it, L in (
                        (GA, BA, ita, NIA),
                        (GB, BB, itb, NIB),
                    ):
                        o = 0
                        while o < L:
                            n = min(1024, L - o)
                            nc.gpsimd.dma_gather(
                                Gt[:, o // 128 : (o + n) // 128, :],
                                aug[view_base:, :],
                                it[:, o // 16 : (o + n) // 16],
                                n,
                                n,
                                EW,
                            )
                            o += n

                for wl in range(SB):
                    w = sbi * SB + wl
                    ow = sbp.tile([WIN, 132], f32, tag="ow")
                    nc.sync.dma_start(out=ow[:], in_=own[w * WIN : (w + 1) * WIN, :])
                    adw = sbp.tile([WIN, 1], f16, tag="adw")
                    nc.scalar.activation(out=adw[:], in_=ow[:, 128:129], func=AF.Copy)

                    adp = psa.tile([128, K], f32, tag="adp")
                    if NO_MATVEC:
                        nc.vector.memset(adp[:], 0.0)
                    else:
                        for c in range(K):
                            j = wl * K + c
                            nc.tensor.matmul(
                                out=adp[:, c : c + 1],
                                lhsT=s2t[:, j * 128 : (j + 1) * 128],
                                rhs=adw[:],
                                start=True,
                                stop=True,
                            )
                    tt = sbp.tile([128, K], f32, tag="tt")
                    nc.vector.tensor_tensor(
                        out=tt[:, 0:CA],
                        in0=GA[:, wl * CA : (wl + 1) * CA, 128],
                        in1=GA[:, wl * CA : (wl + 1) * CA, 129],
                        op=mybir.AluOpType.add,
                    )
                    nc.vector.tensor_tensor(
                        out=tt[:, CA:K],
                        in0=GB[:, wl * CB : (wl + 1) * CB, 128],
                        in1=GB[:, wl * CB : (wl + 1) * CB, 129],
                        op=mybir.AluOpType.add,
                    )
                    t1 = sbp.tile([128, K], f32, tag="t1")
                    nc.vector.tensor_tensor(
                        out=t1[:], in0=tt[:], in1=adp[:], op=mybir.AluOpType.add
                    )
                    g = sbp.tile([128, K], f32, tag="g")
                    nc.scalar.activation(out=g[:], in_=t1[:], func=AF.Tanh, bias=gb_t[:])
                    gs = sbp.tile([128, K], f32, tag="gs")
                    nc.vector.tensor_copy(gs[:], g[:])

                    zp = psz.tile([WIN, 128], f32, tag="zp")
                    for c in range(K):
                        j = wl * K + c
                        if c < CA:
                            Gx, col = GA, wl * CA + c
                        else:
                            Gx, col = GB, wl * CB + (c - CA)
                        sw = iota_f if NO_SW else swp.tile([128, WIN], f16, tag="sw")
                        if not NO_SW:
                            nc.vector.tensor_scalar(
                                out=sw[:],
                                in0=iota_f[:],
                                scalar1=dls[:, j : j + 1],
                                scalar2=gs[:, c : c + 1],
                                op0=mybir.AluOpType.is_equal,
                                op1=mybir.AluOpType.mult,
                            )
                        if not NO_SCATTER:
                            nc.tensor.matmul(
                                out=zp[:],
                                lhsT=sw[:],
                                rhs=Gx[:, col, 0:128],
                                start=(c == 0),
                                stop=(c == K - 1),
                            )
                        elif c == 0:
                            nc.vector.memset(zp[:], 0.0)
                    d1 = sbp.tile([WIN, 128], f32, tag="d1")
                    nc.scalar.activation(
                        out=d1[:], in_=zp[:], func=AF.Copy, scale=ow[:, 129:130]
                    )
                    ot = sbp.tile([WIN, 128], f32, tag="ot")
                    nc.vector.tensor_add(ot[:], d1[:], ow[:, 0:128])
                    nc.sync.dma_start(out=out[w * WIN : (w + 1) * WIN, :], in_=ot[:])
    nc.finalize()
    return nc


def _pack_idx16(flat_rel, n):
    """flat order i -> [i%16, i//16]; replicated 8x across 128 partitions."""
    t = np.zeros((16, n // 16), np.int16)
    t[np.arange(n) % 16, np.arange(n) // 16] = flat_rel
    return np.tile(t, (8, 1))


def _host_prep(edge_index):
    src = edge_index[0].astype(np.int64)
    dst = edge_index[1].astype(np.int64)
    deg = np.bincount(dst, minlength=N).astype(np.int64)

    # degree-balanced window assignment (iterative LPT)
    order = np.argsort(-deg, kind="stable")
    win_of_node = np.empty(N, np.int64)
    pos_of_node = np.empty(N, np.int64)
    load = np.zeros(NWT, np.int64)
    cnt = np.zeros(NWT, np.int64)
    for r0 in range(0, N, NWT):
        grp = order[r0 : r0 + NWT]
        bins = np.argsort(load, kind="stable")[: len(grp)]
        win_of_node[grp] = bins
        pos_of_node[grp] = cnt[bins]
        cnt[bins] += 1
        load[bins] += deg[grp]

    slot_of_node = win_of_node * WIN + pos_of_node
    orig_of_slot = np.full(NSLOT, -1, np.int64)
    orig_of_slot[slot_of_node] = np.arange(N)

    # per-edge attributes
    ew = win_of_node[dst]                # window
    es = slot_of_node[src]               # src slot
    eslab = (es >= SPLIT).astype(np.int64)
    edl = pos_of_node[dst]               # dst position in window

    key = ew * 2 + eslab
    sidx = np.argsort(key, kind="stable")
    key_s = key[sidx]
    counts = np.bincount(key_s, minlength=NWT * 2)
    gstart = np.zeros(NWT * 2, np.int64)
    gstart[1:] = np.cumsum(counts)[:-1]
    qpos = np.arange(E) - gstart[key_s]  # position within (window, slab) group

    cntsA = counts[0::2]
    cntsB = counts[1::2]
    CA = int(np.ceil(cntsA.max() / 128))
    CB = int(np.ceil(cntsB.max() / 128))
    K = CA + CB

    es_s = es[sidx]
    edl_s = edl[sidx]
    ew_s = ew[sidx]
    eslab_s = eslab[sidx]

    core_s = ew_s // NWC
    wloc = ew_s % NWC
    sb_s = wloc // SB
    wl_s = wloc % SB
    p_s = qpos % 128
    c_loc = qpos // 128
    cchunk = np.where(eslab_s == 0, c_loc, CA + c_loc)
    j_s = wl_s * K + cchunk
    relidx = np.where(eslab_s == 0, es_s - BA, es_s - BB)

    NIA = 128 * SB * CA
    NIB = 128 * SB * CB
    per_core = []
    for c in range(NCORES):
        m = core_s == c
        sbv, wlv, pv, clv, jv = sb_s[m], wl_s[m], p_s[m], c_loc[m], j_s[m]
        slabv, relv, dlv, qv = eslab_s[m], relidx[m], edl_s[m], qpos[m]

        dl_arr = np.full((NSB, 128, SB * K), -1.0, np.float32)
        dl_arr[sbv, pv, jv] = dlv
        s2_arr = np.zeros((NSB, 64, SB * K * 128), np.float16)
        s2_arr[sbv, dlv, jv * 128 + pv] = 1.0

        fa = np.zeros((NSB, NIA), np.int64)   # rel idx 0 = valid pad row
        fb = np.zeros((NSB, NIB), np.int64)
        mA = slabv == 0
        fa[sbv[mA], wlv[mA] * CA * 128 + qv[mA]] = relv[mA]
        mB = ~mA
        fb[sbv[mB], wlv[mB] * CB * 128 + qv[mB]] = relv[mB]

        # The Q7 gather drops a call's trailing run of negative indices, so
        # every call's final slot must hold a non-negative index. Swap a
        # non-negative slot from the same (window, slab) group into each
        # static call-tail position.
        def _fix_tails(f, C, joff):
            L = f.shape[1]
            tails = [min(o + 1024, L) - 1 for o in range(0, L, 1024)]
            span = 128 * C
            for s in range(NSB):
                for t in tails:
                    if f[s, t] >= 0:
                        continue
                    wg = t // span
                    g0 = wg * span
                    seg = f[s, g0 : g0 + span]
                    cand = np.nonzero(seg >= 0)[0]
                    assert cand.size, "all-negative gather group"
                    u = g0 + int(cand[-1])
                    p_t, c_t = t % 128, (t // 128) % C
                    p_u, c_u = u % 128, (u // 128) % C
                    j_t = wg * K + joff + c_t
                    j_u = wg * K + joff + c_u
                    f[s, t], f[s, u] = f[s, u], f[s, t]
                    tmp = dl_arr[s, p_t, j_t]
                    dl_arr[s, p_t, j_t] = dl_arr[s, p_u, j_u]
                    dl_arr[s, p_u, j_u] = tmp
                    ct, cu = j_t * 128 + p_t, j_u * 128 + p_u
                    tcol = s2_arr[s, :, ct].copy()
                    s2_arr[s, :, ct] = s2_arr[s, :, cu]
                    s2_arr[s, :, cu] = tcol

        _fix_tails(fa, CA, 0)
        _fix_tails(fb, CB, CA)
        ia_arr = np.stack([_pack_idx16(fa[s], NIA) for s in range(NSB)])
        ib_arr = np.stack([_pack_idx16(fb[s], NIB) for s in range(NSB)])
        per_core.append(dict(ia=ia_arr, ib=ib_arr, dl=dl_arr, s2=s2_arr))

    return dict(
        deg=deg, orig_of_slot=orig_of_slot, slot_of_node=slot_of_node,
        CA=CA, CB=CB, per_core=per_core,
    )


def kernel(x, edge_index, w1, b1, gate_w, gate_b):
    x = np.asarray(x, np.float32)
    edge_index = np.asarray(edge_index)
    w1 = np.asarray(w1, np.float32)
    b1 = np.asarray(b1, np.float32)
    gate_w = np.asarray(gate_w, np.float32)
    gate_b = np.asarray(gate_b, np.float32)

    prep = _host_prep(edge_index)
    CA, CB = prep["CA"], prep["CB"]
    orig_of_slot = prep["orig_of_slot"]
    deg = prep["deg"]

    # per-slot x / deg (zeros for empty slots)
    x_slots = np.zeros((NSLOT, IN_DIM), np.float32)
    deg_slots = np.zeros((NSLOT, 1), np.float32)
    valid = orig_of_slot >= 0
    x_slots[valid] = x[orig_of_slot[valid]]
    deg_slots[valid, 0] = deg[orig_of_slot[valid]]

    if "p1" not in _prog_cache:
        _prog_cache["p1"] = _build_phase1()
    nc1 = _prog_cache["p1"]

    w1T = np.ascontiguousarray(w1.T.reshape(2, 128, HID).astype(np.float16))
    b1c = np.ascontiguousarray(b1[:, None])
    gwT = np.ascontiguousarray(gate_w.reshape(2, HID).T.astype(np.float16))
    in_maps1 = [
        dict(
            xT=np.ascontiguousarray(
                x_slots[c * NPC : (c + 1) * NPC].T.reshape(2, 128, NPC)
            ).astype(np.float16),
            deg=deg_slots[c * NPC : (c + 1) * NPC],
            w1T=w1T, b1=b1c, gwT=gwT,
        )
        for c in range(NCORES)
    ]
    import os

    do_trace = os.environ.get("KERNEL_TRACE", "0") == "1"
    global LAST_EXEC_NS
    LAST_EXEC_NS = [None, None]
    br1 = run_bass_kernel_spmd(nc1, in_maps1, list(range(NCORES)), trace=do_trace)
    r1 = br1.results
    LAST_EXEC_NS[0] = br1.exec_time_ns

    aug_full = np.concatenate([r1[c]["aug"] for c in range(NCORES)], axis=0)

    key2 = ("p2", CA, CB)
    if key2 not in _prog_cache:
        _prog_cache[key2] = _build_phase2(CA, CB)
    nc2 = _prog_cache[key2]

    gbc = np.full((128, 1), float(gate_b[0]), np.float32)
    in_maps2 = [
        dict(
            aug=aug_full,
            own=r1[c]["own"],
            ia=prep["per_core"][c]["ia"],
            ib=prep["per_core"][c]["ib"],
            dl=prep["per_core"][c]["dl"],
            s2=prep["per_core"][c]["s2"],
            gbc=gbc,
        )
        for c in range(NCORES)
    ]
    br2 = run_bass_kernel_spmd(nc2, in_maps2, list(range(NCORES)), trace=do_trace)
    r2 = br2.results
    LAST_EXEC_NS[1] = br2.exec_time_ns

    out_slots = np.concatenate([r2[c]["out"] for c in range(NCORES)], axis=0)
    result = np.empty((N, HID), np.float32)
    result[orig_of_slot[valid]] = out_slots[valid]
    return result

